# revision 15
# baseline (speedup 1.0000x reference)
"""ChemConv Trainium2 kernel (fused-G bf16 streaming version).

Computes, for A=2048 atoms, IN_DEPTH=D=128, OUT_DEPTH=O=128, FILTER_LEN=F=16:

  nc1[a,f,d]  = sum_b conn[a,b,f] * node[b,d]
  combined    = concat([nc1, bond], axis=2)            # (A, F, D+2)
  out[a,o]    = sum_{f,k} combined[a,f,k] * filters[o,f,k]

Sharding: atom rows of conn split across 8 NeuronCores (A/8 = 256 atoms each);
node/filters replicated. No cross-device reduction.

Algebraic fusion: out[o,a] = sum_{b,f} conn[a,b,f] * G[b,f,o]  with
G[b,f,o] = sum_d node[b,d] * filters[o,f,d].  G (2048 x 16 x 128, bf16) is
built ON-CHIP from node/filters (16 chunk matmuls + PSUM->SBUF copies,
~0.5 MB of extra HBM instead of 8 MB for a host-built G), then the whole
kernel is one 32768-deep PSUM accumulation into a single [o, a] tile:
no nc1 intermediate, no per-block casts, and the tail after the last conn
byte is just matmuls + one copy + one DMA.

The kernel is HBM-bound on the conn stream: conn is cast to bf16 on host
(16 MB/core; measured end-to-end rel err ~3.1e-3 vs the 2e-2 gate) and
pre-transposed to [bp, bc, f, a] (b = bc*128 + bp) so every DMA descriptor
is a contiguous 8 KB per-partition line and each matmul RHS is a plain
slice.  Streaming alternates the two HWDGE queues; the final chunk rides
its own buffer so the scheduler hoists its load ahead of the stream tail
(end-of-stream completion semaphores flush late under full HBM load).
Host pre-transposes filters/bond (filtT[d,(f,o)], bfiltT[(f,j),o],
bondT[(f,j),a]) and nodeT[d,b]. Host concats/transposes per-core outputs.
"""

import ml_dtypes
import numpy as np

import concourse.bacc as bacc
import concourse.mybir as mybir
import concourse.tile as tile
from concourse.bass_utils import run_bass_kernel_spmd

A, D, O, F = 2048, 128, 128, 16
NCORES = 8
AL = A // NCORES   # atoms per core = 256
NC_B = 16          # b-chunks (contraction b = bc*128 + bp)
BP = 128           # b per chunk (partition dim)
FO = F * O         # G free size per chunk = 2048
F2 = F * 2

_f32 = mybir.dt.float32
_bf16 = mybir.dt.bfloat16
_bf = ml_dtypes.bfloat16


def _build():
    nc = bacc.Bacc("TRN2", target_bir_lowering=False, debug=False)

    BLK = F * AL  # conn elements per partition per chunk = 4096 (8 KB bf16)

    conn = nc.dram_tensor("conn", [BP, NC_B * BLK], _bf16, kind="ExternalInput")
    nodeT = nc.dram_tensor("nodeT", [D, NC_B * BP], _bf16, kind="ExternalInput")
    filtT = nc.dram_tensor("filtT", [D, FO], _bf16, kind="ExternalInput")
    bfiltT = nc.dram_tensor("bfiltT", [F2, O], _bf16, kind="ExternalInput")
    bondT = nc.dram_tensor("bondT", [F2, AL], _bf16, kind="ExternalInput")
    out = nc.dram_tensor("out", [O, AL], _f32, kind="ExternalOutput")

    NQ = 4  # G-build sub-matmuls per chunk (N=512 each, one PSUM bank)

    with tile.TileContext(nc) as tc:
        with (
            tc.tile_pool(name="sb", bufs=1) as sb,
            tc.tile_pool(name="connp", bufs=6) as connp,
            tc.tile_pool(name="psg", bufs=4, space="PSUM") as psg,
            tc.tile_pool(name="psacc", bufs=1, space="PSUM") as psacc,
        ):
            # nodeT/filtT lead (G-build inputs); small tensors follow
            nodeT_sb = sb.tile([D, NC_B * BP], _bf16)
            nc.sync.dma_start(nodeT_sb[:], nodeT[:])
            filtT_sb = sb.tile([D, FO], _bf16)
            nc.scalar.dma_start(filtT_sb[:], filtT[:])
            bfiltT_sb = sb.tile([F2, O], _bf16)
            nc.scalar.dma_start(bfiltT_sb[:], bfiltT[:])
            bondT_sb = sb.tile([F2, AL], _bf16)
            nc.scalar.dma_start(bondT_sb[:], bondT[:])

            G_sb = sb.tile([BP, NC_B * FO], _bf16)   # 64 KB/partition
            out_sb = sb.tile([O, AL], _f32)
            acc = psacc.tile([O, AL], _f32)

            def g_build(bc):
                # G chunk bc: [bp, (f,o)] = nodeT[:, bc].T @ filtT
                for q in range(NQ):
                    pg = psg.tile([BP, FO // NQ], _f32, tag="pg")
                    nc.tensor.matmul(
                        pg[:],
                        nodeT_sb[:, bc * BP : (bc + 1) * BP],
                        filtT_sb[:, q * (FO // NQ) : (q + 1) * (FO // NQ)],
                        start=True,
                        stop=True,
                    )
                    nc.any.tensor_copy(
                        G_sb[:, bc * FO + q * (FO // NQ) : bc * FO + (q + 1) * (FO // NQ)],
                        pg[:],
                    )

            def main_chunk(bc, ct):
                for f in range(F):
                    nc.tensor.matmul(
                        acc[:],
                        G_sb[:, bc * FO + f * O : bc * FO + (f + 1) * O],
                        ct[:, f * AL : (f + 1) * AL],
                        start=(bc == 0 and f == 0),
                        stop=False,
                    )

            # interleave G-build with the main stream: round g builds G
            # chunk g and consumes conn chunk g-2, so PE work tracks both
            # the G dependencies and the conn deliveries
            cts = {}
            for bc in range(NC_B):
                if bc < NC_B - 1:
                    ct = connp.tile([BP, BLK], _bf16, tag="conn")
                    eng = nc.sync if bc % 2 == 0 else nc.scalar
                    eng.dma_start(ct[:], conn[:, bc * BLK : (bc + 1) * BLK])
                else:
                    # own buffer -> scheduler hoists this load early; the
                    # stream tail then gates only on chunk 14
                    ct = sb.tile([BP, BLK], _bf16)
                    nc.sync.dma_start(ct[:], conn[:, bc * BLK : (bc + 1) * BLK])
                cts[bc] = ct
                g_build(bc)
                if bc >= 2:
                    main_chunk(bc - 2, cts.pop(bc - 2))
            main_chunk(NC_B - 2, cts.pop(NC_B - 2))
            main_chunk(NC_B - 1, cts.pop(NC_B - 1))

            # bond term closes the accumulation
            nc.tensor.matmul(acc[:], bfiltT_sb[:], bondT_sb[:], start=False, stop=True)
            nc.vector.tensor_copy(out_sb[:], acc[:])
            nc.scalar.dma_start(out[:], out_sb[:])

    nc.compile()
    return nc


def _in_maps(node_property_tensor, connectivity_tensor, bond_property_tensor, filters):
    node = np.asarray(node_property_tensor, dtype=np.float32)
    conn = np.asarray(connectivity_tensor, dtype=np.float32)
    bond = np.asarray(bond_property_tensor, dtype=np.float32)
    filt = np.asarray(filters, dtype=np.float32)

    # conn[(c, a), (bc, bp), f] -> per core [bp, bc, f, a], bf16
    conn_r = np.ascontiguousarray(
        conn.reshape(NCORES, AL, NC_B, BP, F).transpose(0, 3, 2, 4, 1)
    ).astype(_bf)
    conn_r = conn_r.reshape(NCORES, BP, NC_B * F * AL)

    nodeT = np.ascontiguousarray(node.T).astype(_bf)                   # [d, b]
    filtT = np.ascontiguousarray(
        filt[:, :, :D].transpose(2, 1, 0)
    ).astype(_bf).reshape(D, F * O)                                    # [d, (f, o)]
    bfiltT = np.ascontiguousarray(
        filt[:, :, D:].transpose(1, 2, 0)
    ).astype(_bf).reshape(F2, O)                                       # [(f, j), o]
    bondT = np.ascontiguousarray(
        bond.reshape(NCORES, AL, F, 2).transpose(0, 2, 3, 1)
    ).astype(_bf).reshape(NCORES, F2, AL)                              # [(f, j), a]

    maps = []
    for c in range(NCORES):
        maps.append(
            {
                "conn": conn_r[c],
                "nodeT": nodeT,
                "filtT": filtT,
                "bfiltT": bfiltT,
                "bondT": bondT[c],
            }
        )
    return maps


def _enable_tracing():
    """Install the NTFF profile hook (missing antenv.axon_hooks shim) and
    neuter the artifact upload (zero-egress container). Profiling only —
    never touched on the plain kernel() path."""
    import sys
    import types

    try:
        import antenv.axon_hooks  # noqa: F401
    except ImportError:
        from trn_agent_boot.trn_boot import _ntff_profile_via_ctypes

        hook = _ntff_profile_via_ctypes("/opt/axon/libaxon_pjrt.so")
        mod = types.ModuleType("antenv.axon_hooks")
        mod._hook = hook
        mod.get_axon_ntff_profile_hook = lambda: mod._hook
        mod.set_axon_ntff_profile_hook = lambda h: setattr(mod, "_hook", h)
        sys.modules["antenv.axon_hooks"] = mod
        import antenv

        antenv.axon_hooks = mod

    import concourse.bass_utils as _bu

    _bu.upload_artifacts = lambda tmpdir: tmpdir


def run(
    node_property_tensor,
    connectivity_tensor,
    bond_property_tensor,
    filters,
    trace=False,
):
    """Run the sharded kernel; returns (full (A, O) output, exec_time_ns|None)."""
    if trace:
        _enable_tracing()
    nc = _build()
    maps = _in_maps(
        node_property_tensor, connectivity_tensor, bond_property_tensor, filters
    )
    res = run_bass_kernel_spmd(nc, maps, core_ids=list(range(NCORES)), trace=trace)
    parts = [res.results[c]["out"] for c in range(NCORES)]  # each (O, AL)
    full = np.concatenate(parts, axis=1).T  # (A, O)
    return np.ascontiguousarray(full, dtype=np.float32), res.exec_time_ns


def kernel(
    node_property_tensor, connectivity_tensor, bond_property_tensor, filters
) -> np.ndarray:
    out, _ = run(
        node_property_tensor, connectivity_tensor, bond_property_tensor, filters
    )
    return out


# revision 21
# speedup vs baseline: 1.0360x; 1.0360x over previous
"""ChemConv Trainium2 kernel (fused-G bf16 streaming version).

Computes, for A=2048 atoms, IN_DEPTH=D=128, OUT_DEPTH=O=128, FILTER_LEN=F=16:

  nc1[a,f,d]  = sum_b conn[a,b,f] * node[b,d]
  combined    = concat([nc1, bond], axis=2)            # (A, F, D+2)
  out[a,o]    = sum_{f,k} combined[a,f,k] * filters[o,f,k]

Sharding: atom rows of conn split across 8 NeuronCores (A/8 = 256 atoms each);
node/filters replicated. No cross-device reduction.

Algebraic fusion: out[o,a] = sum_{b,f} conn[a,b,f] * G[b,f,o]  with
G[b,f,o] = sum_d node[b,d] * filters[o,f,d].  G (2048 x 16 x 128, bf16) is
built ON-CHIP from node/filters (16 chunk matmuls + PSUM->SBUF copies,
~0.5 MB of extra HBM instead of 8 MB for a host-built G), then the whole
kernel is one 32768-deep PSUM accumulation into a single [o, a] tile:
no nc1 intermediate, no per-block casts, and the tail after the last conn
byte is just matmuls + one copy + one DMA.

The kernel is HBM-bound on the conn stream: conn is cast to bf16 on host
(16 MB/core; measured end-to-end rel err ~3.1e-3 vs the 2e-2 gate) and
pre-transposed to [bp, bc, f, a] (b = bc*128 + bp) so every DMA descriptor
is a contiguous 8 KB per-partition line and each matmul RHS is a plain
slice.  Streaming alternates the two HWDGE queues; the final chunk rides
its own buffer so the scheduler hoists its load ahead of the stream tail
(end-of-stream completion semaphores flush late under full HBM load).
Host pre-transposes filters/bond (filtT[d,(f,o)], bfiltT[(f,j),o],
bondT[(f,j),a]) and nodeT[d,b]. Host concats/transposes per-core outputs.
"""

import ml_dtypes
import numpy as np

import concourse.bacc as bacc
import concourse.mybir as mybir
import concourse.tile as tile
from concourse.bass_utils import run_bass_kernel_spmd

A, D, O, F = 2048, 128, 128, 16
NCORES = 8
AL = A // NCORES   # atoms per core = 256
NC_B = 16          # b-chunks (contraction b = bc*128 + bp)
BP = 128           # b per chunk (partition dim)
FO = F * O         # G free size per chunk = 2048
F2 = F * 2

_f32 = mybir.dt.float32
_bf16 = mybir.dt.bfloat16
_bf = ml_dtypes.bfloat16


def _build():
    nc = bacc.Bacc("TRN2", target_bir_lowering=False, debug=False)

    BLK = F * AL  # conn elements per partition per chunk = 4096 (8 KB bf16)

    conn = nc.dram_tensor("conn", [BP, NC_B * BLK], _bf16, kind="ExternalInput")
    nodeT = nc.dram_tensor("nodeT", [D, NC_B * BP], _bf16, kind="ExternalInput")
    filtT = nc.dram_tensor("filtT", [D, FO], _bf16, kind="ExternalInput")
    bfiltT = nc.dram_tensor("bfiltT", [F2, O], _bf16, kind="ExternalInput")
    bondT = nc.dram_tensor("bondT", [F2, AL], _bf16, kind="ExternalInput")
    out = nc.dram_tensor("out", [O, AL], _f32, kind="ExternalOutput")

    NQ = 4  # G-build sub-matmuls per chunk (N=512 each, one PSUM bank)

    with tile.TileContext(nc) as tc:
        with (
            tc.tile_pool(name="sb", bufs=1) as sb,
            tc.tile_pool(name="connp", bufs=6) as connp,
            tc.tile_pool(name="psg", bufs=4, space="PSUM") as psg,
            tc.tile_pool(name="psacc", bufs=1, space="PSUM") as psacc,
        ):
            # nodeT/filtT lead (G-build inputs); nodeT split into two tiles
            # so the first G chunks start before the whole of nodeT lands
            NSPLIT = 4 * BP
            nodeT_a = sb.tile([D, NSPLIT], _bf16)
            nc.sync.dma_start(nodeT_a[:], nodeT[:, 0:NSPLIT])
            filtT_sb = sb.tile([D, FO], _bf16)
            nc.scalar.dma_start(filtT_sb[:], filtT[:])
            nodeT_b = sb.tile([D, NC_B * BP - NSPLIT], _bf16)
            nc.sync.dma_start(nodeT_b[:], nodeT[:, NSPLIT : NC_B * BP])
            bfiltT_sb = sb.tile([F2, O], _bf16)
            nc.scalar.dma_start(bfiltT_sb[:], bfiltT[:])
            bondT_sb = sb.tile([F2, AL], _bf16)
            nc.scalar.dma_start(bondT_sb[:], bondT[:])

            G_sb = sb.tile([BP, NC_B * FO], _bf16)   # 64 KB/partition
            out_sb = sb.tile([O, AL], _f32)
            acc = psacc.tile([O, AL], _f32)

            def g_build(bc):
                # G chunk bc: [bp, (f,o)] = nodeT[:, bc].T @ filtT
                if bc < 4:
                    nt = nodeT_a[:, bc * BP : (bc + 1) * BP]
                else:
                    nt = nodeT_b[:, (bc - 4) * BP : (bc - 3) * BP]
                for q in range(NQ):
                    pg = psg.tile([BP, FO // NQ], _f32, tag="pg")
                    nc.tensor.matmul(
                        pg[:],
                        nt,
                        filtT_sb[:, q * (FO // NQ) : (q + 1) * (FO // NQ)],
                        start=True,
                        stop=True,
                    )
                    dst = G_sb[
                        :, bc * FO + q * (FO // NQ) : bc * FO + (q + 1) * (FO // NQ)
                    ]
                    if q % 2 == 0:
                        nc.vector.tensor_copy(dst, pg[:])
                    else:
                        nc.scalar.copy(dst, pg[:])

            def main_chunk(bc, ct):
                for f in range(F):
                    nc.tensor.matmul(
                        acc[:],
                        G_sb[:, bc * FO + f * O : bc * FO + (f + 1) * O],
                        ct[:, f * AL : (f + 1) * AL],
                        start=(bc == 0 and f == 0),
                        stop=False,
                    )

            # prefetch the final chunk right behind the aux tensors: its
            # matmuls then run off resident data at the stream tail
            ct_last = sb.tile([BP, BLK], _bf16)
            nc.sync.dma_start(ct_last[:], conn[:, (NC_B - 1) * BLK : NC_B * BLK])

            # interleave G-build with the main stream: round g builds G
            # chunk g and consumes conn chunk g-2, so PE work tracks both
            # the G dependencies and the conn deliveries
            cts = {NC_B - 1: ct_last}
            for bc in range(NC_B - 1):
                if bc < NC_B - 2:
                    ct = connp.tile([BP, BLK], _bf16, tag="conn")
                    eng = nc.sync if bc % 2 == 0 else nc.scalar
                    eng.dma_start(ct[:], conn[:, bc * BLK : (bc + 1) * BLK])
                else:
                    # last streamed chunk rides both queues as halves so the
                    # end-of-stream drain is not single-queue latency-bound
                    ct = connp.tile([BP, BLK], _bf16, tag="conn")
                    nc.sync.dma_start(
                        ct[:, 0 : BLK // 2], conn[:, bc * BLK : bc * BLK + BLK // 2]
                    )
                    nc.scalar.dma_start(
                        ct[:, BLK // 2 : BLK],
                        conn[:, bc * BLK + BLK // 2 : (bc + 1) * BLK],
                    )
                cts[bc] = ct
                g_build(bc)
                if bc >= 2:
                    main_chunk(bc - 2, cts.pop(bc - 2))
            g_build(NC_B - 1)
            main_chunk(NC_B - 3, cts.pop(NC_B - 3))
            main_chunk(NC_B - 2, cts.pop(NC_B - 2))
            main_chunk(NC_B - 1, cts.pop(NC_B - 1))

            # bond term closes the accumulation
            nc.tensor.matmul(acc[:], bfiltT_sb[:], bondT_sb[:], start=False, stop=True)
            nc.vector.tensor_copy(out_sb[:], acc[:])
            nc.scalar.dma_start(out[:], out_sb[:])

    nc.compile()
    return nc


def _in_maps(node_property_tensor, connectivity_tensor, bond_property_tensor, filters):
    node = np.asarray(node_property_tensor, dtype=np.float32)
    conn = np.asarray(connectivity_tensor, dtype=np.float32)
    bond = np.asarray(bond_property_tensor, dtype=np.float32)
    filt = np.asarray(filters, dtype=np.float32)

    # conn[(c, a), (bc, bp), f] -> per core [bp, bc, f, a], bf16
    conn_r = np.ascontiguousarray(
        conn.reshape(NCORES, AL, NC_B, BP, F).transpose(0, 3, 2, 4, 1)
    ).astype(_bf)
    conn_r = conn_r.reshape(NCORES, BP, NC_B * F * AL)

    nodeT = np.ascontiguousarray(node.T).astype(_bf)                   # [d, b]
    filtT = np.ascontiguousarray(
        filt[:, :, :D].transpose(2, 1, 0)
    ).astype(_bf).reshape(D, F * O)                                    # [d, (f, o)]
    bfiltT = np.ascontiguousarray(
        filt[:, :, D:].transpose(1, 2, 0)
    ).astype(_bf).reshape(F2, O)                                       # [(f, j), o]
    bondT = np.ascontiguousarray(
        bond.reshape(NCORES, AL, F, 2).transpose(0, 2, 3, 1)
    ).astype(_bf).reshape(NCORES, F2, AL)                              # [(f, j), a]

    maps = []
    for c in range(NCORES):
        maps.append(
            {
                "conn": conn_r[c],
                "nodeT": nodeT,
                "filtT": filtT,
                "bfiltT": bfiltT,
                "bondT": bondT[c],
            }
        )
    return maps


def _enable_tracing():
    """Install the NTFF profile hook (missing antenv.axon_hooks shim) and
    neuter the artifact upload (zero-egress container). Profiling only —
    never touched on the plain kernel() path."""
    import sys
    import types

    try:
        import antenv.axon_hooks  # noqa: F401
    except ImportError:
        from trn_agent_boot.trn_boot import _ntff_profile_via_ctypes

        hook = _ntff_profile_via_ctypes("/opt/axon/libaxon_pjrt.so")
        mod = types.ModuleType("antenv.axon_hooks")
        mod._hook = hook
        mod.get_axon_ntff_profile_hook = lambda: mod._hook
        mod.set_axon_ntff_profile_hook = lambda h: setattr(mod, "_hook", h)
        sys.modules["antenv.axon_hooks"] = mod
        import antenv

        antenv.axon_hooks = mod

    import concourse.bass_utils as _bu

    _bu.upload_artifacts = lambda tmpdir: tmpdir


def run(
    node_property_tensor,
    connectivity_tensor,
    bond_property_tensor,
    filters,
    trace=False,
):
    """Run the sharded kernel; returns (full (A, O) output, exec_time_ns|None)."""
    if trace:
        _enable_tracing()
    nc = _build()
    maps = _in_maps(
        node_property_tensor, connectivity_tensor, bond_property_tensor, filters
    )
    res = run_bass_kernel_spmd(nc, maps, core_ids=list(range(NCORES)), trace=trace)
    parts = [res.results[c]["out"] for c in range(NCORES)]  # each (O, AL)
    full = np.concatenate(parts, axis=1).T  # (A, O)
    return np.ascontiguousarray(full, dtype=np.float32), res.exec_time_ns


def kernel(
    node_property_tensor, connectivity_tensor, bond_property_tensor, filters
) -> np.ndarray:
    out, _ = run(
        node_property_tensor, connectivity_tensor, bond_property_tensor, filters
    )
    return out
